# revision 17
# baseline (speedup 1.0000x reference)
"""Trainium2 Bass kernel for nn_MultiHeadAttention_44281112822190.

8 NeuronCores, pure data parallelism over the 8192 (b,s) rows: core c takes
rows [c*1024, (c+1)*1024). No collectives; host shards inputs / reassembles.

Math notes (same as the 584us baseline):
  - RoPE cancels in the per-position head-head scores ((Rq).(Rk) = q.k), so
    the kernel skips it entirely.
  - The reference's transpose(0,2,1,3).reshape scramble maps
    out[b, h*128 + s//16, (s%16)*128 + d] = att_out[b, s, h, d]; each
    scrambled row stays inside one core's shard.

Differences vs the 584us baseline (all scheduling / engine-placement —
numerics unchanged: fp16 operands, fp32 PSUM):
  - x is DMA'd in four 256-position quarters and the QKV projections loop
    quarter-inner, so the PE starts ~4us after launch instead of waiting
    ~22us for the full 4MB x transfer.
  - The per-pair attention transposes (att^T and v^T, 4 per 16-position
    pair) moved from the PE (where each cost ~107ns of LDWEIGHTS-bound
    time) to the DMA xbar transpose engine. PE attention work per pair is
    now just 2 score matmuls + 2 att@v matmuls.
  - Attention is split into a score stage (scores->exp->mask+rowsum->
    recip->normalize->attT transpose) and an apply stage (vT transpose,
    att@v matmuls, scatter). The V projection runs quarter-by-quarter
    (wv streamed 4x) and score/apply stages for earlier quarters
    interleave into its matmul stream, so the softmax chain latency hides
    under projection matmuls instead of stalling the PE.
  - PSUM->SBUF copybacks (with bias) rotate across Vector/Scalar/GpSimd.
  - Output is stored fp16 (rel err budget is 2e-2; this adds ~5e-4).

Per-unit emission: every (t2, half) projection unit is 16 256-col matmuls
(~1.75us); attached to each V/out-proj unit is at most one score stage and
one apply stage, lagged so every cross-engine dependency has >=1 unit of
slack before the PE needs it.
"""

import sys

sys.path.insert(0, "/opt/trn_rl_repo")

import numpy as np

import concourse.bacc as bacc
import concourse.mybir as mybir
import concourse.tile as tile
from concourse.bass_utils import run_bass_kernel_spmd

F32 = mybir.dt.float32
F16 = mybir.dt.float16
AF = mybir.ActivationFunctionType
ALU = mybir.AluOpType

B, S, E, H, D = 4, 2048, 2048, 16, 128
NCORES = 8
SCALE = 1.0 / float(np.sqrt(D))

_CACHE = {}
LAST_EXEC_NS = None


def _build():
    nc = bacc.Bacc(trn_type="TRN2", target_bir_lowering=False)

    xt = nc.dram_tensor("xt", [16, 128, 1024], F16, kind="ExternalInput")
    wqt = nc.dram_tensor("wqt", [E, E], F16, kind="ExternalInput")
    wkt = nc.dram_tensor("wkt", [E, E], F16, kind="ExternalInput")
    wvt = nc.dram_tensor("wvt", [E, E], F16, kind="ExternalInput")
    wot = nc.dram_tensor("wot", [E, E], F16, kind="ExternalInput")
    bqt = nc.dram_tensor("bqt", [128, 16], F32, kind="ExternalInput")
    bkt = nc.dram_tensor("bkt", [128, 16], F32, kind="ExternalInput")
    bvt = nc.dram_tensor("bvt", [128, 16], F32, kind="ExternalInput")
    bot = nc.dram_tensor("bot", [128, 16], F32, kind="ExternalInput")
    mask01 = nc.dram_tensor("mask01", [128, 128], F16, kind="ExternalInput")
    out = nc.dram_tensor("out", [16, 128, 1024], F16, kind="ExternalOutput")

    with tile.TileContext(nc) as tc:
        with (
            tc.tile_pool(name="const", bufs=1) as cp,
            tc.tile_pool(name="xp", bufs=1) as xp,
            tc.tile_pool(name="qkv", bufs=1) as qkvp,
            tc.tile_pool(name="aop", bufs=1) as aop,
            tc.tile_pool(name="wp", bufs=3) as wp,
            tc.tile_pool(name="sc", bufs=7) as scp,
            tc.tile_pool(name="atp", bufs=20) as atp,
            tc.tile_pool(name="vtp", bufs=4) as vtp,
            tc.tile_pool(name="obp", bufs=4) as obp,
            tc.tile_pool(name="pp", bufs=4, space="PSUM") as pp,
            tc.tile_pool(name="pa", bufs=2, space="PSUM") as pa,
            tc.tile_pool(name="po", bufs=2, space="PSUM") as pop,
        ):
            mask_sb = cp.tile([128, 128], F16, tag="mask")
            nc.sync.dma_start(mask_sb[:], mask01[:, :])
            bias_sb = {}
            for name, t_ in (("bq", bqt), ("bk", bkt), ("bv", bvt), ("bo", bot)):
                b_sb = cp.tile([128, 16], F32, tag=name, name=f"b_{name}")
                nc.sync.dma_start(b_sb[:], t_[:, :])
                bias_sb[name] = b_sb

            def w_dma(wdram, t2):
                wtile = wp.tile([128, 16, 256], F16, tag="w", name="wt")
                nc.sync.dma_start(
                    wtile[:],
                    wdram[:, t2 * 256 : (t2 + 1) * 256].rearrange(
                        "(k p) c -> p k c", p=128
                    ),
                )
                return wtile

            # first weight tile before the x quarters so the PE can start
            # as soon as (wq[0], xq[0]) land instead of after all of x.
            wq_first = w_dma(wqt, 0)

            xq = []
            for qtr in range(4):
                xc = xp.tile([128, 16, 256], F16, tag=f"xq{qtr}", name=f"xq{qtr}")
                nc.sync.dma_start(
                    xc[:],
                    xt[:, :, qtr * 256 : (qtr + 1) * 256].rearrange("k p s -> p k s"),
                )
                xq.append(xc)

            qb = qkvp.tile([128, 1024, 16], F16, tag="qb")
            kb = qkvp.tile([128, 1024, 16], F16, tag="kb")
            vb = qkvp.tile([128, 1024, 16], F16, tag="vb")
            attO_h = [
                aop.tile([128, 16, 256], F16, tag=f"attO{q}", name=f"attO{q}")
                for q in range(4)
            ]

            # engine rotation for psum->sbuf copybacks (with per-partition bias)
            cb_state = [0]

            def copyback(dst, src, bias_ap):
                # GPSIMD cannot read PSUM; rotate Vector/Scalar only.
                e = cb_state[0] % 2
                cb_state[0] += 1
                if e == 0:
                    nc.vector.tensor_scalar_add(dst, src, bias_ap)
                else:
                    nc.scalar.add(dst, src, bias_ap)

            # ---------------- attention stages ----------------
            attT_tiles = {}
            vT_tiles = {}
            e2_tiles = {}

            def score_stage(p):
                s0 = 16 * p
                ga = pa.tile([128, 256], F32, tag="ga", name="ga")
                for j in range(2):
                    nc.tensor.matmul(
                        ga[:, 128 * j : 128 * j + 128],
                        qb[:, s0 + 8 * j : s0 + 8 * j + 8, :],
                        kb[:, s0 + 8 * j : s0 + 8 * j + 8, :],
                        start=True, stop=True,
                    )
                e2 = scp.tile([128, 2, 128], F16, tag="e2", name="e2")
                nc.scalar.activation(
                    e2[:], ga[:].rearrange("p (j c) -> p j c", j=2), AF.Exp,
                    scale=SCALE,
                )
                den = scp.tile([128, 2], F32, tag="den", name="den")
                for j in range(2):
                    nc.vector.scalar_tensor_tensor(
                        e2[:, j, :], e2[:, j, :], 1.0, mask_sb[:],
                        ALU.bypass, ALU.mult,
                        accum_out=den[:, j : j + 1],
                    )
                rec = scp.tile([128, 2], F32, tag="rec", name="rec")
                nc.vector.reciprocal(rec[:], den[:])
                nc.gpsimd.tensor_tensor(
                    e2[:], e2[:],
                    rec[:].unsqueeze(2).to_broadcast([128, 2, 128]), ALU.mult,
                )
                e2_tiles[p] = e2

            def atr_stage(p):
                # issued >=4 units after score_stage(p): by now the softmax
                # chain is done, so this SP-queue DMA issue never blocks the
                # weight stream behind it (head-of-line).
                e2 = e2_tiles.pop(p)
                aT = atp.tile([128, 2, 128], F16, tag="attT", name="attT")
                for j in range(2):
                    nc.sync.dma_start_transpose(aT[:, j, :], e2[:, j, :])
                attT_tiles[p] = aT

            def tpre_stage(p):
                s0 = 16 * p
                vT = vtp.tile([128, 2, 128], F16, tag="vT", name="vT")
                for j in range(2):
                    nc.sync.dma_start_transpose(
                        vT[:, j, :], vb[:, s0 + 8 * j : s0 + 8 * j + 8, :]
                    )
                vT_tiles[p] = vT

            def tmm_stage(p):
                vT = vT_tiles.pop(p)
                aT = attT_tiles.pop(p)
                ps = pop.tile([128, 256], F32, tag="po", name="po")
                for j in range(2):
                    nc.tensor.matmul(
                        ps[:, 128 * j : 128 * j + 128],
                        vT[:, j, :], aT[:, j, :],
                        start=True, stop=True,
                    )
                u_hi, u_lo = p // 16, p % 16
                dst = attO_h[u_hi][:].rearrange(
                    "p (g2 i) (u h) -> p g2 i u h", g2=2, h=16
                )[:, :, :, u_lo, :]
                nc.vector.tensor_copy(dst, ps[:])

            # stage scheduler state: lag tmm one unit behind tpre, and lag
            # the attT transpose issues 4 units behind their score stage
            tpre_done = []
            scored_pending = []

            def attach(score_p=None, tpre_p=None):
                if tpre_p is not None and tpre_p < 64:
                    if tpre_done:
                        tmm_stage(tpre_done.pop(0))
                    tpre_stage(tpre_p)
                    tpre_done.append(tpre_p)
                if scored_pending and (
                    score_p is None or len(scored_pending) >= 4
                ):
                    atr_stage(scored_pending.pop(0))
                if score_p is not None and score_p < 64:
                    score_stage(score_p)
                    scored_pending.append(score_p)

            def flush_tmm():
                while tpre_done:
                    tmm_stage(tpre_done.pop(0))

            # ---------------- Q and K projections ----------------
            for wdram, bias, dst in ((wqt, "bq", qb), (wkt, "bk", kb)):
                for t2 in range(8):
                    wtile = (
                        wq_first
                        if (wdram is wqt and t2 == 0)
                        else w_dma(wdram, t2)
                    )
                    for half in range(2):
                        t = 2 * t2 + half
                        for qpair in range(2):
                            ps = pp.tile([128, 512], F32, tag="pp", name="ps")
                            for qi in range(2):
                                qtr = 2 * qpair + qi
                                reg = ps[:, qi * 256 : (qi + 1) * 256]
                                for k in range(16):
                                    nc.tensor.matmul(
                                        reg,
                                        wtile[:, k, half * 128 : half * 128 + 128],
                                        xq[qtr][:, k, :],
                                        start=(k == 0), stop=(k == 15),
                                    )
                                copyback(
                                    dst[:, qtr * 256 : (qtr + 1) * 256, t],
                                    reg, bias_sb[bias][:, t : t + 1],
                                )

            # ---------------- V projection (quarter-outer) + attention ----
            for vq in range(4):
                for t2 in range(8):
                    wtile = w_dma(wvt, t2)
                    ps = pp.tile([128, 512], F32, tag="pp", name="ps")
                    for half in range(2):
                        t = 2 * t2 + half
                        u = t2 * 2 + half
                        reg = ps[:, half * 256 : (half + 1) * 256]
                        for k in range(16):
                            nc.tensor.matmul(
                                reg,
                                wtile[:, k, half * 128 : half * 128 + 128],
                                xq[vq][:, k, :],
                                start=(k == 0), stop=(k == 15),
                            )
                        copyback(
                            vb[:, vq * 256 : (vq + 1) * 256, t],
                            reg, bias_sb["bv"][:, t : t + 1],
                        )
                        if vq == 0:
                            attach(score_p=u)
                        else:
                            attach(score_p=16 * vq + u, tpre_p=16 * (vq - 1) + u)

            # ---------------- output projection ----------------
            for q in range(4):
                for t2 in range(8):
                    wtile = w_dma(wot, t2)
                    ps = pp.tile([128, 512], F32, tag="pp", name="ps")
                    for half in range(2):
                        t = 2 * t2 + half
                        u = t2 * 2 + half
                        reg = ps[:, half * 256 : (half + 1) * 256]
                        for sl in range(16):
                            nc.tensor.matmul(
                                reg,
                                wtile[:, sl, half * 128 : half * 128 + 128],
                                attO_h[q][:, sl, :],
                                start=(sl == 0), stop=(sl == 15),
                            )
                        if q == 0:
                            attach(tpre_p=48 + u)
                        elif q == 1 and u == 0:
                            flush_tmm()
                        ob = obp.tile([128, 256], F16, tag="ob", name="ob")
                        copyback(ob[:], reg, bias_sb["bo"][:, t : t + 1])
                        nc.gpsimd.dma_start(
                            out[t, :, q * 256 : q * 256 + 256], ob[:]
                        )

    nc.compile()
    return nc


def _get_nc():
    if "nc" not in _CACHE:
        _CACHE["nc"] = _build()
    return _CACHE["nc"]


def make_in_maps(inputs):
    x = np.ascontiguousarray(np.asarray(inputs["x"], dtype=np.float32))
    ws = {k: np.asarray(inputs[k], dtype=np.float32) for k in ("wq", "wk", "wv", "wo")}
    bs = {k: np.asarray(inputs[k], dtype=np.float32) for k in ("bq", "bk", "bv", "bo")}

    xf = x.reshape(B * S, E)
    f16 = lambda a: np.ascontiguousarray(a).astype(np.float16)
    btile = lambda b: np.ascontiguousarray(b.reshape(16, 128).T)
    ii = np.arange(128) // 16
    mask01 = (ii[:, None] == ii[None, :]).astype(np.float16)
    common = {
        "wqt": f16(ws["wq"].T), "wkt": f16(ws["wk"].T),
        "wvt": f16(ws["wv"].T), "wot": f16(ws["wo"].T),
        "bqt": btile(bs["bq"]), "bkt": btile(bs["bk"]),
        "bvt": btile(bs["bv"]), "bot": btile(bs["bo"]),
        "mask01": mask01,
    }
    in_maps = []
    for c in range(NCORES):
        xt_c = f16(xf[c * 1024 : (c + 1) * 1024].T).reshape(16, 128, 1024)
        in_maps.append({"xt": xt_c, **common})
    return in_maps


def assemble(results):
    out = np.empty((B, S, E), np.float32)
    for c in range(NCORES):
        O = results[c]["out"].astype(np.float32)  # [16 t, 128 p, 1024]; col = u*16+h
        Oc = O.reshape(E, 64, 16)  # [j, u, h]
        tgt = out[c // 2].reshape(16, 128, E)
        v0 = (c % 2) * 64
        tgt[:, v0 : v0 + 64, :] = Oc.transpose(2, 1, 0)
    return out


def kernel(**inputs):
    global LAST_EXEC_NS
    nc = _get_nc()
    res = run_bass_kernel_spmd(nc, make_in_maps(inputs), core_ids=list(range(NCORES)))
    LAST_EXEC_NS = res.exec_time_ns
    return assemble(res.results)


# revision 23
# speedup vs baseline: 1.6658x; 1.6658x over previous
"""Trainium2 Bass kernel for nn_MultiHeadAttention_44281112822190.

8 NeuronCores, pure data parallelism over the 8192 (b,s) rows: core c takes
rows [c*1024, (c+1)*1024). No collectives; host shards inputs / reassembles.

Math notes (same as the 584us baseline):
  - RoPE cancels in the per-position head-head scores ((Rq).(Rk) = q.k), so
    the kernel skips it entirely.
  - The reference's transpose(0,2,1,3).reshape scramble maps
    out[b, h*128 + s//16, (s%16)*128 + d] = att_out[b, s, h, d]; each
    scrambled row stays inside one core's shard.

Differences vs the 584us baseline (all scheduling / engine-placement —
numerics unchanged: fp16 operands, fp32 PSUM):
  - x is DMA'd in four 256-position quarters and the QKV projections loop
    quarter-inner, so the PE starts ~4us after launch instead of waiting
    ~22us for the full 4MB x transfer.
  - The per-pair attention transposes (att^T and v^T, 4 per 16-position
    pair) moved from the PE (where each cost ~107ns of LDWEIGHTS-bound
    time) to the DMA xbar transpose engine. PE attention work per pair is
    now just 2 score matmuls + 2 att@v matmuls.
  - Attention is split into a score stage (scores->exp->mask+rowsum->
    recip->normalize->attT transpose) and an apply stage (vT transpose,
    att@v matmuls, scatter). The V projection runs quarter-by-quarter
    (wv streamed 4x) and score/apply stages for earlier quarters
    interleave into its matmul stream, so the softmax chain latency hides
    under projection matmuls instead of stalling the PE.
  - PSUM->SBUF copybacks (with bias) rotate across Vector/Scalar/GpSimd.
  - Output is stored fp16 (rel err budget is 2e-2; this adds ~5e-4).

Per-unit emission: every (t2, half) projection unit is 16 256-col matmuls
(~1.75us); attached to each V/out-proj unit is at most one score stage and
one apply stage, lagged so every cross-engine dependency has >=1 unit of
slack before the PE needs it.
"""

import sys

sys.path.insert(0, "/opt/trn_rl_repo")

import numpy as np

import concourse.bacc as bacc
import concourse.mybir as mybir
import concourse.tile as tile
from concourse.bass_utils import run_bass_kernel_spmd

F32 = mybir.dt.float32
F16 = mybir.dt.float16
AF = mybir.ActivationFunctionType
ALU = mybir.AluOpType

B, S, E, H, D = 4, 2048, 2048, 16, 128
NCORES = 8
SCALE = 1.0 / float(np.sqrt(D))

_CACHE = {}
LAST_EXEC_NS = None


def _build():
    nc = bacc.Bacc(trn_type="TRN2", target_bir_lowering=False)

    xt = nc.dram_tensor("xt", [16, 128, 1024], F16, kind="ExternalInput")
    wqt = nc.dram_tensor("wqt", [E, E], F16, kind="ExternalInput")
    wkt = nc.dram_tensor("wkt", [E, E], F16, kind="ExternalInput")
    wvt = nc.dram_tensor("wvt", [E, E], F16, kind="ExternalInput")
    wot = nc.dram_tensor("wot", [E, E], F16, kind="ExternalInput")
    bqt = nc.dram_tensor("bqt", [128, 16], F32, kind="ExternalInput")
    bkt = nc.dram_tensor("bkt", [128, 16], F32, kind="ExternalInput")
    bvt = nc.dram_tensor("bvt", [128, 16], F32, kind="ExternalInput")
    bot = nc.dram_tensor("bot", [128, 16], F32, kind="ExternalInput")
    mask01 = nc.dram_tensor("mask01", [128, 128], F16, kind="ExternalInput")
    ident = nc.dram_tensor("ident", [128, 128], F16, kind="ExternalInput")
    out = nc.dram_tensor("out", [16, 128, 1024], F16, kind="ExternalOutput")

    with tile.TileContext(nc) as tc:
        with (
            tc.tile_pool(name="const", bufs=1) as cp,
            tc.tile_pool(name="xp", bufs=1) as xp,
            tc.tile_pool(name="qkv", bufs=1) as qkvp,
            tc.tile_pool(name="aop", bufs=1) as aop,
            tc.tile_pool(name="wp", bufs=3) as wp,
            tc.tile_pool(name="sc", bufs=4) as scp,
            tc.tile_pool(name="trs", bufs=3) as trs,
            tc.tile_pool(name="obp", bufs=4) as obp,
            tc.tile_pool(name="pp", bufs=3, space="PSUM") as pp,
            tc.tile_pool(name="pa", bufs=3, space="PSUM") as pa,
            tc.tile_pool(name="pb", bufs=2, space="PSUM") as pb,
        ):
            mask_sb = cp.tile([128, 128], F16, tag="mask")
            nc.sync.dma_start(mask_sb[:], mask01[:, :])
            id_sb = cp.tile([128, 128], F16, tag="id")
            nc.sync.dma_start(id_sb[:], ident[:, :])
            bias_sb = {}
            for name, t_ in (("bq", bqt), ("bk", bkt), ("bv", bvt), ("bo", bot)):
                b_sb = cp.tile([128, 16], F32, tag=name, name=f"b_{name}")
                nc.sync.dma_start(b_sb[:], t_[:, :])
                bias_sb[name] = b_sb

            def w_dma(wdram, t2):
                wtile = wp.tile([128, 16, 256], F16, tag="w", name="wt")
                nc.sync.dma_start(
                    wtile[:],
                    wdram[:, t2 * 256 : (t2 + 1) * 256].rearrange(
                        "(k p) c -> p k c", p=128
                    ),
                )
                return wtile

            # first weight tile before the x quarters so the PE can start
            # as soon as (wq[0], xq[0]) land instead of after all of x.
            wq_first = w_dma(wqt, 0)

            xq = []
            for qtr in range(4):
                xc = xp.tile([128, 16, 256], F16, tag=f"xq{qtr}", name=f"xq{qtr}")
                nc.sync.dma_start(
                    xc[:],
                    xt[:, :, qtr * 256 : (qtr + 1) * 256].rearrange("k p s -> p k s"),
                )
                xq.append(xc)

            qb = qkvp.tile([128, 1024, 16], F16, tag="qb")
            kb = qkvp.tile([128, 1024, 16], F16, tag="kb")
            vb = qkvp.tile([128, 1024, 16], F16, tag="vb")
            attO_h = [
                aop.tile([128, 16, 256], F16, tag=f"attO{q}", name=f"attO{q}")
                for q in range(4)
            ]

            # engine rotation for psum->sbuf copybacks (with per-partition bias)
            cb_state = [0]

            def copyback(dst, src, bias_ap):
                # GPSIMD cannot read PSUM; rotate Vector/Scalar only.
                e = cb_state[0] % 2
                cb_state[0] += 1
                if e == 0:
                    nc.vector.tensor_scalar_add(dst, src, bias_ap)
                else:
                    nc.scalar.add(dst, src, bias_ap)

            # ---------------- attention stages ----------------
            e2_tiles = {}
            trsb_tiles = {}

            def score_stage(p):
                s0 = 16 * p
                ga = pa.tile([128, 256], F32, tag="ga", name="ga")
                for j in range(2):
                    nc.tensor.matmul(
                        ga[:, 128 * j : 128 * j + 128],
                        qb[:, s0 + 8 * j : s0 + 8 * j + 8, :],
                        kb[:, s0 + 8 * j : s0 + 8 * j + 8, :],
                        start=True, stop=True,
                    )
                e2 = scp.tile([128, 2, 128], F16, tag="e2", name="e2", bufs=20)
                nc.scalar.activation(
                    e2[:], ga[:].rearrange("p (j c) -> p j c", j=2), AF.Exp,
                    scale=SCALE,
                )
                den = scp.tile([128, 2], F32, tag="den", name="den")
                for j in range(2):
                    nc.vector.scalar_tensor_tensor(
                        e2[:, j, :], e2[:, j, :], 1.0, mask_sb[:],
                        ALU.bypass, ALU.mult,
                        accum_out=den[:, j : j + 1],
                    )
                rec = scp.tile([128, 2], F32, tag="rec", name="rec")
                nc.vector.reciprocal(rec[:], den[:])
                nc.gpsimd.tensor_tensor(
                    e2[:], e2[:],
                    rec[:].unsqueeze(2).to_broadcast([128, 2, 128]), ALU.mult,
                )
                e2_tiles[p] = e2

            def t1_stage(p):
                # PE transposes of att (e2, normalized) and the v slab into
                # PSUM; scalar copies them to SBUF for the next unit's T2.
                s0 = 16 * p
                e2 = e2_tiles.pop(p)
                tr = pb.tile([128, 512], F16, tag="tr", name="tr")
                for j in range(2):
                    nc.tensor.transpose(
                        tr[:, 128 * j : 128 * j + 128], e2[:, j, :], id_sb[:]
                    )
                    nc.tensor.transpose(
                        tr[:, 256 + 128 * j : 384 + 128 * j],
                        vb[:, s0 + 8 * j : s0 + 8 * j + 8, :], id_sb[:],
                    )
                trsb = trs.tile([128, 512], F16, tag="trsb", name="trsb")
                nc.scalar.copy(trsb[:], tr[:])
                trsb_tiles[p] = trsb

            def t2_stage(p):
                trsb = trsb_tiles.pop(p)
                ps = pa.tile([128, 256], F32, tag="ga", name="po")
                for j in range(2):
                    nc.tensor.matmul(
                        ps[:, 128 * j : 128 * j + 128],
                        trsb[:, 256 + 128 * j : 384 + 128 * j],
                        trsb[:, 128 * j : 128 * j + 128],
                        start=True, stop=True,
                    )
                u_hi, u_lo = p // 16, p % 16
                dst = attO_h[u_hi][:].rearrange(
                    "p (g2 i) (u h) -> p g2 i u h", g2=2, h=16
                )[:, :, :, u_lo, :]
                nc.vector.tensor_copy(dst, ps[:])

            # lag T2 one unit behind T1 so the PSUM->SBUF copy of the
            # transposes never blocks the PE
            t1_done = []

            def attach(score_p=None, t_p=None):
                if t_p is not None and t_p < 64:
                    if t1_done:
                        t2_stage(t1_done.pop(0))
                    t1_stage(t_p)
                    t1_done.append(t_p)
                if score_p is not None and score_p < 64:
                    score_stage(score_p)

            def flush_t2():
                while t1_done:
                    t2_stage(t1_done.pop(0))

            # ---------------- Q and K projections ----------------
            for wdram, bias, dst in ((wqt, "bq", qb), (wkt, "bk", kb)):
                for t2 in range(8):
                    wtile = (
                        wq_first
                        if (wdram is wqt and t2 == 0)
                        else w_dma(wdram, t2)
                    )
                    for half in range(2):
                        t = 2 * t2 + half
                        for qpair in range(2):
                            ps = pp.tile([128, 512], F32, tag="pp", name="ps")
                            for qi in range(2):
                                qtr = 2 * qpair + qi
                                reg = ps[:, qi * 256 : (qi + 1) * 256]
                                for k in range(16):
                                    nc.tensor.matmul(
                                        reg,
                                        wtile[:, k, half * 128 : half * 128 + 128],
                                        xq[qtr][:, k, :],
                                        start=(k == 0), stop=(k == 15),
                                    )
                                copyback(
                                    dst[:, qtr * 256 : (qtr + 1) * 256, t],
                                    reg, bias_sb[bias][:, t : t + 1],
                                )

            # ---------------- V projection (quarter-outer) + attention ----
            for vq in range(4):
                for t2 in range(8):
                    wtile = w_dma(wvt, t2)
                    ps = pp.tile([128, 512], F32, tag="pp", name="ps")
                    for half in range(2):
                        t = 2 * t2 + half
                        u = t2 * 2 + half
                        reg = ps[:, half * 256 : (half + 1) * 256]
                        for k in range(16):
                            nc.tensor.matmul(
                                reg,
                                wtile[:, k, half * 128 : half * 128 + 128],
                                xq[vq][:, k, :],
                                start=(k == 0), stop=(k == 15),
                            )
                        copyback(
                            vb[:, vq * 256 : (vq + 1) * 256, t],
                            reg, bias_sb["bv"][:, t : t + 1],
                        )
                        if vq == 0:
                            attach(score_p=u)
                        else:
                            attach(score_p=16 * vq + u, t_p=16 * (vq - 1) + u)

            # ---------------- output projection ----------------
            for q in range(4):
                for t2 in range(8):
                    wtile = w_dma(wot, t2)
                    ps = pp.tile([128, 512], F32, tag="pp", name="ps")
                    for half in range(2):
                        t = 2 * t2 + half
                        u = t2 * 2 + half
                        reg = ps[:, half * 256 : (half + 1) * 256]
                        for sl in range(16):
                            nc.tensor.matmul(
                                reg,
                                wtile[:, sl, half * 128 : half * 128 + 128],
                                attO_h[q][:, sl, :],
                                start=(sl == 0), stop=(sl == 15),
                            )
                        if q == 0:
                            attach(t_p=48 + u)
                        elif q == 1 and u == 0:
                            flush_t2()
                        ob = obp.tile([128, 256], F16, tag="ob", name="ob")
                        copyback(ob[:], reg, bias_sb["bo"][:, t : t + 1])
                        nc.gpsimd.dma_start(
                            out[t, :, q * 256 : q * 256 + 256], ob[:]
                        )

    nc.compile()
    return nc


def _get_nc():
    if "nc" not in _CACHE:
        _CACHE["nc"] = _build()
    return _CACHE["nc"]


def make_in_maps(inputs):
    x = np.ascontiguousarray(np.asarray(inputs["x"], dtype=np.float32))
    ws = {k: np.asarray(inputs[k], dtype=np.float32) for k in ("wq", "wk", "wv", "wo")}
    bs = {k: np.asarray(inputs[k], dtype=np.float32) for k in ("bq", "bk", "bv", "bo")}

    xf = x.reshape(B * S, E)
    f16 = lambda a: np.ascontiguousarray(a).astype(np.float16)
    btile = lambda b: np.ascontiguousarray(b.reshape(16, 128).T)
    ii = np.arange(128) // 16
    mask01 = (ii[:, None] == ii[None, :]).astype(np.float16)
    common = {
        "wqt": f16(ws["wq"].T), "wkt": f16(ws["wk"].T),
        "wvt": f16(ws["wv"].T), "wot": f16(ws["wo"].T),
        "bqt": btile(bs["bq"]), "bkt": btile(bs["bk"]),
        "bvt": btile(bs["bv"]), "bot": btile(bs["bo"]),
        "mask01": mask01, "ident": np.eye(128, dtype=np.float16),
    }
    in_maps = []
    for c in range(NCORES):
        xt_c = f16(xf[c * 1024 : (c + 1) * 1024].T).reshape(16, 128, 1024)
        in_maps.append({"xt": xt_c, **common})
    return in_maps


def assemble(results):
    out = np.empty((B, S, E), np.float32)
    for c in range(NCORES):
        O = results[c]["out"].astype(np.float32)  # [16 t, 128 p, 1024]; col = u*16+h
        Oc = O.reshape(E, 64, 16)  # [j, u, h]
        tgt = out[c // 2].reshape(16, 128, E)
        v0 = (c % 2) * 64
        tgt[:, v0 : v0 + 64, :] = Oc.transpose(2, 1, 0)
    return out


def kernel(**inputs):
    global LAST_EXEC_NS
    nc = _get_nc()
    res = run_bass_kernel_spmd(nc, make_in_maps(inputs), core_ids=list(range(NCORES)))
    LAST_EXEC_NS = res.exec_time_ns
    return assemble(res.results)
